# revision 23
# baseline (speedup 1.0000x reference)
"""Causal self-attention (B=4, S=2048, D=1024, H=16, hd=64) on 8 TRN2 cores.

Sharding: core c = (batch b = c//2, head-group g = c%2); each core computes
8 heads for one batch. Out-projection partials are summed on host (the only
cross-shard reduction).

Schedule: the score->exp->AV pipeline is the ACT-bound critical path; all
other PE work (next-pair QKV chains, v-projection, out-projection) is woven
into it as fine-grained "filler" half-units popped from a global queue, one
per (score, AV) slot, so the PE never drains (keeps the tensor engine at its
max p-state) and the ACT engine always has exp work queued.  Causal masks
are applied in-place on the exp output by gpsimd.affine_select (no mask
tiles, no DVE mask-mul).  PSUM: 2x[128,1024] score tiles (4 banks),
3x[65,512] rotating AV accumulators (3 banks; 3-deep so ib boundaries don't
stall), 1x[128,512] shared by all filler chains.  Output is written bf16
and summed on host in f32.
"""
import sys
import os
from collections import deque

sys.path.insert(0, "/opt/trn_rl_repo")

import numpy as np
import ml_dtypes
from contextlib import ExitStack

S = 2048
D = 1024
HL = 8          # heads per core
HD = 64
PAIRS = 4       # head pairs per core
NIB = 4         # i-blocks of 512
N_CORES = 8

_CACHE = {}
LAST_EXEC_TIME_NS = None


def _build():
    import concourse.tile as tile
    import concourse.mybir as mybir
    from concourse import bacc

    bf = mybir.dt.bfloat16
    f32 = mybir.dt.float32
    EXP = mybir.ActivationFunctionType.Exp
    GE = mybir.AluOpType.is_ge

    nc = bacc.Bacc("TRN2", target_bir_lowering=False, debug=False,
                   num_devices=N_CORES)
    xT_d = nc.dram_tensor("xT", [D, S], bf, kind="ExternalInput").ap()
    wqkvT_d = nc.dram_tensor("wqkvT", [D, 3 * 512], bf,
                             kind="ExternalInput").ap()
    woutT_d = nc.dram_tensor("woutT", [512, D], bf, kind="ExternalInput").ap()
    out_d = nc.dram_tensor("out", [S, D], bf, kind="ExternalOutput").ap()

    with tile.TileContext(nc) as tc, ExitStack() as ctx:
        sb = ctx.enter_context(tc.tile_pool(name="sb", bufs=1))
        mm = ctx.enter_context(tc.tile_pool(name="mm", bufs=2, space="PSUM"))
        av = ctx.enter_context(tc.tile_pool(name="av", bufs=3, space="PSUM"))
        ps5 = ctx.enter_context(tc.tile_pool(name="ps5", bufs=1,
                                             space="PSUM"))
        pp = ctx.enter_context(tc.tile_pool(name="pp", bufs=8))
        rsp = ctx.enter_context(tc.tile_pool(name="rsp", bufs=4))
        bcsp = ctx.enter_context(tc.tile_pool(name="bcsp", bufs=4))
        osbp = ctx.enter_context(tc.tile_pool(name="osbp", bufs=6))

        # ---- persistent SBUF tiles -------------------------------------
        xt = [sb.tile([128, S], bf, tag=f"xt{d}", name=f"xt{d}")
              for d in range(8)]
        wqkv = [sb.tile([128, 1536], bf, tag=f"wqkv{d}", name=f"wqkv{d}")
                for d in range(8)]
        wout = [sb.tile([128, D], bf, tag=f"wout{c}", name=f"wout{c}")
                for c in range(4)]
        qT = [sb.tile([128, S], bf, tag=f"qT{p}", name=f"qT{p}")
              for p in range(PAIRS)]
        kT = [sb.tile([128, S], bf, tag=f"kT{p}", name=f"kT{p}")
              for p in range(PAIRS)]
        vaug = [sb.tile([128, HL, HD + 1], bf, tag=f"vaug{s}",
                        name=f"vaug{s}") for s in range(16)]
        attnT = [sb.tile([128, S], bf, tag=f"attnT{p}", name=f"attnT{p}")
                 for p in range(PAIRS)]

        # ---- DMAs, priority-ordered -------------------------------------
        # wqkvT host layout: [q0|k0|q1|k1|q2|k2|q3|k3|v]: q(p) at 256p,
        # k(p) at 256p+128, v at 1024.  DMA cost is ~35ns per partition-row
        # descriptor per queue, so order by first use, one DMA per queue.
        # 1) pair-0 q+k (one 256-col slice) and x cols 0:512: 16 parallel
        for dc in range(8):
            nc.sync.dma_start(wqkv[dc][:, 0:256],
                              wqkvT_d[128 * dc:128 * (dc + 1), 0:256])
            nc.sync.dma_start(xt[dc][:, 0:512],
                              xT_d[128 * dc:128 * (dc + 1), 0:512])
        # 2) v weight columns + x cols 512:1024
        for dc in range(8):
            nc.sync.dma_start(wqkv[dc][:, 1024:1536],
                              wqkvT_d[128 * dc:128 * (dc + 1), 1024:1536])
            nc.sync.dma_start(xt[dc][:, 512:1024],
                              xT_d[128 * dc:128 * (dc + 1), 512:1024])
        # 3) rest of x
        for dc in range(8):
            nc.sync.dma_start(xt[dc][:, 1024:1536],
                              xT_d[128 * dc:128 * (dc + 1), 1024:1536])
            nc.sync.dma_start(xt[dc][:, 1536:2048],
                              xT_d[128 * dc:128 * (dc + 1), 1536:2048])
        # 4) q/k weights for pairs 1..3
        for dc in range(8):
            nc.sync.dma_start(wqkv[dc][:, 256:1024],
                              wqkvT_d[128 * dc:128 * (dc + 1), 256:1024])
        # 5) out-proj weights (needed only in pair 3)
        for c in range(4):
            nc.sync.dma_start(wout[c][:], woutT_d[128 * c:128 * (c + 1), :])
        # ones column for the softmax-sum row of the AV matmul
        for s in range(16):
            nc.gpsimd.memset(vaug[s][:, :, 64:65], 1.0)

        # PE warm-up while input DMAs land: ~3.5us of junk matmuls ramps
        # the tensor engine to its max p-state before the first real chain
        warm = sb.tile([128, 512], bf, tag="warm", name="warm")
        nc.gpsimd.memset(warm[:], 0.0)
        for r in range(2):
            wps = ps5.tile([128, 512], f32, tag="ps5", name=f"warm{r}")
            for i in range(8):
                nc.tensor.matmul(wps[:], lhsT=warm[:, 0:128], rhs=warm[:],
                                 start=(i == 0), stop=(i == 7))

        # ---- filler-unit queue ------------------------------------------
        queue = deque()          # (uid, closure)
        remaining = {}           # uid -> halves left to emit

        def push_unit(uid, halves):
            remaining[uid] = len(halves)
            for h in halves:
                queue.append((uid, h))

        def pop_one():
            if queue:
                uid, h = queue.popleft()
                h()
                remaining[uid] -= 1

        def drain_until(uid):
            while remaining.get(uid, 0) > 0:
                u2, h = queue.popleft()
                h()
                remaining[u2] -= 1

        def drain_all():
            while queue:
                pop_one()

        # ---- unit builders ----------------------------------------------
        def qk_unit(pair, qk, sc):
            # qk: 0 -> q (wqkv cols 256*pair), 1 -> k (cols 256*pair+128)
            nb = 2 * pair + qk
            dest = qT[pair] if qk == 0 else kT[pair]
            st = {}

            def h1():
                ps = ps5.tile([128, 512], f32, tag="ps5",
                              name=f"qk{pair}_{qk}_{sc}")
                st["ps"] = ps
                for dc in range(4):
                    nc.tensor.matmul(
                        ps[:], lhsT=wqkv[dc][:, 128 * nb:128 * (nb + 1)],
                        rhs=xt[dc][:, 512 * sc:512 * (sc + 1)],
                        start=(dc == 0), stop=False)

            def h2():
                ps = st["ps"]
                for dc in range(4, 8):
                    nc.tensor.matmul(
                        ps[:], lhsT=wqkv[dc][:, 128 * nb:128 * (nb + 1)],
                        rhs=xt[dc][:, 512 * sc:512 * (sc + 1)],
                        start=False, stop=(dc == 7))
                nc.vector.tensor_copy(dest[:, 512 * sc:512 * (sc + 1)],
                                      ps[:])
            return [h1, h2]

        def v_unit(sblk):
            st = {}

            def h1():
                ps = ps5.tile([128, 512], f32, tag="ps5", name=f"v{sblk}")
                st["ps"] = ps
                for dc in range(4):
                    nc.tensor.matmul(
                        ps[:], lhsT=xt[dc][:, 128 * sblk:128 * (sblk + 1)],
                        rhs=wqkv[dc][:, 1024:1536],
                        start=(dc == 0), stop=False)

            def h2():
                ps = st["ps"]
                for dc in range(4, 8):
                    nc.tensor.matmul(
                        ps[:], lhsT=xt[dc][:, 128 * sblk:128 * (sblk + 1)],
                        rhs=wqkv[dc][:, 1024:1536],
                        start=False, stop=(dc == 7))
                nc.vector.tensor_copy(
                    vaug[sblk][:, :, 0:64],
                    ps[:].rearrange("p (h d) -> p h d", h=HL))
            return [h1, h2]

        DRAIN = {"on": False}

        def op_unit(sblk):
            st = {}

            def half(eh):
                def h():
                    if eh == 0:
                        st["osb"] = osbp.tile([128, D], bf, tag="osbp",
                                              name=f"osb{sblk}")
                    osb = st["osb"]
                    if DRAIN["on"]:
                        # scores are done; ping-pong through the idle mm
                        # pool so drain ops pipeline instead of serializing
                        # on the single ps5 bank
                        ps = mm.tile([128, 512], f32, tag="mm",
                                     name=f"op{sblk}_{eh}")
                    else:
                        ps = ps5.tile([128, 512], f32, tag="ps5",
                                      name=f"op{sblk}_{eh}")
                    for cc in range(4):
                        nc.tensor.matmul(
                            ps[:],
                            lhsT=attnT[cc][:, 128 * sblk:128 * (sblk + 1)],
                            rhs=wout[cc][:, 512 * eh:512 * (eh + 1)],
                            start=(cc == 0), stop=(cc == 3))
                    nc.vector.tensor_copy(osb[:, 512 * eh:512 * (eh + 1)],
                                          ps[:])
                    if eh == 1:
                        # partition-split x2 per col-half: 64-row DMAs
                        for q in range(2):
                            for ph in range(2):
                                nc.sync.dma_start(
                                    out_d[128 * sblk + 64 * ph:
                                          128 * sblk + 64 * (ph + 1),
                                          512 * q:512 * (q + 1)],
                                    osb[64 * ph:64 * (ph + 1),
                                        512 * q:512 * (q + 1)])
                return h
            return [half(0), half(1)]

        # ---- attention emitters -----------------------------------------
        def emit_score(pair, ib, jb):
            off = max(0, 128 * (jb - 4 * ib))
            s2 = mm.tile([128, 1024], f32, tag="mm",
                         name=f"s2_{pair}{ib}{jb}")
            for h01 in range(2):
                r0, r1 = 64 * h01, 64 * (h01 + 1)
                nc.tensor.matmul(
                    s2[:, 512 * h01 + off:512 * (h01 + 1)],
                    lhsT=kT[pair][r0:r1, 128 * jb:128 * (jb + 1)],
                    rhs=qT[pair][r0:r1, 512 * ib + off:512 * (ib + 1)],
                    start=True, stop=True)
            pX = pp.tile([128, 1024], bf, tag="pp", name=f"pX{pair}{ib}{jb}")
            s3 = s2[:].rearrange("p (h i) -> p h i", h=2)
            p3 = pX[:].rearrange("p (h i) -> p h i", h=2)
            nc.scalar.activation(p3[:, :, off:512], s3[:, :, off:512],
                                 EXP, scale=0.125)
            if jb >= 4 * ib:
                # in-place causal wedge: keep where i_rel - j >= 0.  Only
                # the first 128 columns past `off` can be masked (j < 128),
                # so restrict the select to that window.
                nc.gpsimd.affine_select(
                    out=p3[:, :, off:off + 128], in_=p3[:, :, off:off + 128],
                    compare_op=GE, fill=0.0, base=0, channel_multiplier=-1,
                    pattern=[[0, 2], [1, 128]])
            return pX

        def emit_av(pair, ib, jb, pX, oA, oB):
            off = max(0, 128 * (jb - 4 * ib))
            n_jb = 4 * (ib + 1)
            for h01, oX in ((0, oA), (1, oB)):
                nc.tensor.matmul(
                    oX[:, off:512],
                    lhsT=vaug[jb][:, 2 * pair + h01, :],
                    rhs=pX[:, 512 * h01 + off:512 * (h01 + 1)],
                    start=(jb == 0), stop=(jb == n_jb - 1))

        def emit_norm(pair, ib, oA, oB):
            for h01, oX in ((0, oA), (1, oB)):
                tmp = rsp.tile([1, 512], f32, tag="rtmp",
                               name=f"rt{pair}{ib}{h01}")
                nc.vector.tensor_copy(tmp[:], oX[64:65, :])
                rs = rsp.tile([1, 512], f32, tag="rsp",
                              name=f"rs{pair}{ib}{h01}")
                nc.vector.reciprocal_approx_fast(rs[:], tmp[:])
                bcs = bcsp.tile([64, 512], f32, tag="bcsp",
                                name=f"bcs{pair}{ib}{h01}")
                nc.gpsimd.partition_broadcast(bcs[:], rs[:])
                nc.vector.tensor_mul(
                    attnT[pair][64 * h01:64 * (h01 + 1),
                                512 * ib:512 * (ib + 1)],
                    oX[0:64, :], bcs[:])

        # ---- build the global filler queue ------------------------------
        # pushed-per-pair so later (ACT-bound) pairs keep some PE filler:
        #   startup: qk(0,*,0);  pair0: v + qk(0,sc>0) + qk(1,sc0)
        #   pair1: qk(1,sc>0) + qk(2,all);  pair2: qk(3,all);  pair3: ops
        def push_qk(pair, sc):
            push_unit(("qk", pair, 0, sc), qk_unit(pair, 0, sc))
            push_unit(("qk", pair, 1, sc), qk_unit(pair, 1, sc))

        push_qk(0, 0)
        pair_pushes = {
            0: lambda: ([push_unit(("v", s), v_unit(s)) for s in range(4)],
                        [(push_qk(0, sc),
                          [push_unit(("v", s), v_unit(s))
                           for s in range(4 * sc, 4 * sc + 4)])
                         for sc in range(1, 4)],
                        push_qk(1, 0)),
            1: lambda: ([push_qk(1, sc) for sc in range(1, 4)],
                        [push_qk(2, sc) for sc in range(4)]),
            2: lambda: [push_qk(3, sc) for sc in range(4)],
            3: lambda: None,
        }

        # ---- main emission: 4 pairs, score->exp->AV with weaving --------
        for pair in range(PAIRS):
            pair_pushes[pair]()
            # pair 3 runs ibs reversed so each ib's out-proj units weave
            # into the next (smaller) ib instead of trailing the kernel
            ib_order = [3, 2, 1, 0] if pair == 3 else [0, 1, 2, 3]
            for ib in ib_order:
                drain_until(("qk", pair, 0, ib))
                n_jb = 4 * (ib + 1)
                oA = av.tile([65, 512], f32, tag="av", name=f"oA{pair}{ib}")
                oB = av.tile([65, 512], f32, tag="av", name=f"oB{pair}{ib}")
                # AV(jb) trails score(jb+2) so the exp latency is always
                # hidden and the PE never drains waiting on ACT
                pend = deque()
                for jb in range(n_jb):
                    drain_until(("qk", pair, 1, jb // 4))
                    if pair == 0:
                        drain_until(("v", jb))
                    pX = emit_score(pair, ib, jb)
                    if len(pend) >= 3:
                        j0, p0 = pend.popleft()
                        emit_av(pair, ib, j0, p0, oA, oB)
                    pop_one()
                    pend.append((jb, pX))
                while pend:
                    j0, p0 = pend.popleft()
                    emit_av(pair, ib, j0, p0, oA, oB)
                emit_norm(pair, ib, oA, oB)
                if pair == 3:
                    for sblk in range(4 * ib, 4 * ib + 4):
                        push_unit(("op", sblk), op_unit(sblk))
        DRAIN["on"] = True
        drain_all()

    nc.compile()
    return nc


def _get_nc():
    if "nc" not in _CACHE:
        _CACHE["nc"] = _build()
    return _CACHE["nc"]


def _shard_inputs(x, w_qkv, w_out):
    bf = ml_dtypes.bfloat16
    in_maps = []
    for c in range(N_CORES):
        b, g = divmod(c, 2)
        xT = np.ascontiguousarray(x[b].T).astype(bf)
        wq = w_qkv[512 * g:512 * (g + 1)]
        wk = w_qkv[1024 + 512 * g:1024 + 512 * (g + 1)]
        wv = w_qkv[2048 + 512 * g:2048 + 512 * (g + 1)]
        # device column order: [q0|k0|q1|k1|q2|k2|q3|k3|v] (128 cols each)
        parts = []
        for p in range(4):
            parts.append(wq[128 * p:128 * (p + 1)])
            parts.append(wk[128 * p:128 * (p + 1)])
        parts.append(wv)
        wqkvT = np.ascontiguousarray(
            np.concatenate(parts, axis=0).T).astype(bf)
        woutT = np.ascontiguousarray(w_out[:, 512 * g:512 * (g + 1)].T
                                     ).astype(bf)
        in_maps.append({"xT": xT, "wqkvT": wqkvT, "woutT": woutT})
    return in_maps


def kernel(x, w_qkv, w_out):
    global LAST_EXEC_TIME_NS
    from concourse.bass_utils import run_bass_kernel_spmd

    nc = _get_nc()
    in_maps = _shard_inputs(np.asarray(x, dtype=np.float32),
                            np.asarray(w_qkv, dtype=np.float32),
                            np.asarray(w_out, dtype=np.float32))
    trace = bool(int(os.environ.get("KBENCH_TRACE", "0")))
    res = run_bass_kernel_spmd(nc, in_maps, list(range(N_CORES)), trace=trace)
    LAST_EXEC_TIME_NS = res.exec_time_ns
    out = np.empty((4, S, D), dtype=np.float32)
    for b in range(4):
        out[b] = (res.results[2 * b]["out"].astype(np.float32)
                  + res.results[2 * b + 1]["out"].astype(np.float32))
    return out


# revision 25
# speedup vs baseline: 1.0594x; 1.0594x over previous
"""Causal self-attention (B=4, S=2048, D=1024, H=16, hd=64) on 8 TRN2 cores.

Sharding: core c = (batch b = c//2, head-group g = c%2); each core computes
8 heads for one batch. Out-projection partials are summed on host (the only
cross-shard reduction).

Schedule: the score->exp->AV pipeline is the ACT-bound critical path; all
other PE work (next-pair QKV chains, v-projection, out-projection) is woven
into it as fine-grained "filler" half-units popped from a global queue, one
per (score, AV) slot, so the PE never drains (keeps the tensor engine at its
max p-state) and the ACT engine always has exp work queued.  Causal masks
are applied in-place on the exp output by gpsimd.affine_select (no mask
tiles, no DVE mask-mul).  PSUM: 2x[128,1024] score tiles (4 banks),
3x[65,512] rotating AV accumulators (3 banks; 3-deep so ib boundaries don't
stall), 1x[128,512] shared by all filler chains.  Output is written bf16
and summed on host in f32.
"""
import sys
import os
from collections import deque

sys.path.insert(0, "/opt/trn_rl_repo")

import numpy as np
import ml_dtypes
from contextlib import ExitStack

S = 2048
D = 1024
HL = 8          # heads per core
HD = 64
PAIRS = 4       # head pairs per core
NIB = 4         # i-blocks of 512
N_CORES = 8

_CACHE = {}
LAST_EXEC_TIME_NS = None


def _build():
    import concourse.tile as tile
    import concourse.mybir as mybir
    from concourse import bacc

    bf = mybir.dt.bfloat16
    f32 = mybir.dt.float32
    EXP = mybir.ActivationFunctionType.Exp
    GE = mybir.AluOpType.is_ge

    nc = bacc.Bacc("TRN2", target_bir_lowering=False, debug=False,
                   num_devices=N_CORES)
    xT_d = nc.dram_tensor("xT", [D, S], bf, kind="ExternalInput").ap()
    wqkvT_d = nc.dram_tensor("wqkvT", [D, 3 * 512], bf,
                             kind="ExternalInput").ap()
    woutT_d = nc.dram_tensor("woutT", [512, D], bf, kind="ExternalInput").ap()
    out_d = nc.dram_tensor("out", [S, D], bf, kind="ExternalOutput").ap()

    with tile.TileContext(nc) as tc, ExitStack() as ctx:
        sb = ctx.enter_context(tc.tile_pool(name="sb", bufs=1))
        mm = ctx.enter_context(tc.tile_pool(name="mm", bufs=2, space="PSUM"))
        av = ctx.enter_context(tc.tile_pool(name="av", bufs=3, space="PSUM"))
        ps5 = ctx.enter_context(tc.tile_pool(name="ps5", bufs=1,
                                             space="PSUM"))
        pp = ctx.enter_context(tc.tile_pool(name="pp", bufs=8))
        rsp = ctx.enter_context(tc.tile_pool(name="rsp", bufs=4))
        bcsp = ctx.enter_context(tc.tile_pool(name="bcsp", bufs=4))
        osbp = ctx.enter_context(tc.tile_pool(name="osbp", bufs=6))

        # ---- persistent SBUF tiles -------------------------------------
        xt = [sb.tile([128, S], bf, tag=f"xt{d}", name=f"xt{d}")
              for d in range(8)]
        wqkv = [sb.tile([128, 1536], bf, tag=f"wqkv{d}", name=f"wqkv{d}")
                for d in range(8)]
        wout = [sb.tile([128, D], bf, tag=f"wout{c}", name=f"wout{c}")
                for c in range(4)]
        qT = [sb.tile([128, S], bf, tag=f"qT{p}", name=f"qT{p}")
              for p in range(PAIRS)]
        kT = [sb.tile([128, S], bf, tag=f"kT{p}", name=f"kT{p}")
              for p in range(PAIRS)]
        vaug = [sb.tile([128, HL, HD + 1], bf, tag=f"vaug{s}",
                        name=f"vaug{s}") for s in range(16)]
        attnT = [sb.tile([128, S], bf, tag=f"attnT{p}", name=f"attnT{p}")
                 for p in range(PAIRS)]

        # ---- DMAs, priority-ordered -------------------------------------
        # wqkvT host layout: [q0|k0|q1|k1|q2|k2|q3|k3|v]: q(p) at 256p,
        # k(p) at 256p+128, v at 1024.  DMA cost is ~35ns per partition-row
        # descriptor per queue, so order by first use, one DMA per queue.
        # 1) pair-0 q+k (one 256-col slice) and x cols 0:512: 16 parallel
        for dc in range(8):
            nc.sync.dma_start(wqkv[dc][:, 0:256],
                              wqkvT_d[128 * dc:128 * (dc + 1), 0:256])
            nc.sync.dma_start(xt[dc][:, 0:512],
                              xT_d[128 * dc:128 * (dc + 1), 0:512])
        # 2) v weight columns + x cols 512:1024
        for dc in range(8):
            nc.sync.dma_start(wqkv[dc][:, 1024:1536],
                              wqkvT_d[128 * dc:128 * (dc + 1), 1024:1536])
            nc.sync.dma_start(xt[dc][:, 512:1024],
                              xT_d[128 * dc:128 * (dc + 1), 512:1024])
        # 3) rest of x
        for dc in range(8):
            nc.sync.dma_start(xt[dc][:, 1024:1536],
                              xT_d[128 * dc:128 * (dc + 1), 1024:1536])
            nc.sync.dma_start(xt[dc][:, 1536:2048],
                              xT_d[128 * dc:128 * (dc + 1), 1536:2048])
        # 4) q/k weights for pairs 1..3
        for dc in range(8):
            nc.sync.dma_start(wqkv[dc][:, 256:1024],
                              wqkvT_d[128 * dc:128 * (dc + 1), 256:1024])
        # 5) out-proj weights (needed only in pair 3)
        for c in range(4):
            nc.sync.dma_start(wout[c][:], woutT_d[128 * c:128 * (c + 1), :])
        # PE warm-up while input DMAs land: ~3.5us of junk matmuls ramps
        # the tensor engine to its max p-state before the first real chain
        # (memset first so the Pool queue doesn't delay it)
        warm = sb.tile([128, 512], bf, tag="warm", name="warm")
        nc.gpsimd.memset(warm[:], 0.0)
        for r in range(2):
            wps = ps5.tile([128, 512], f32, tag="ps5", name=f"warm{r}")
            for i in range(8):
                nc.tensor.matmul(wps[:], lhsT=warm[:, 0:128], rhs=warm[:],
                                 start=(i == 0), stop=(i == 7))
        # ones column for the softmax-sum row of the AV matmul
        for s in range(16):
            nc.gpsimd.memset(vaug[s][:, :, 64:65], 1.0)

        # ---- filler-unit queue ------------------------------------------
        queue = deque()          # (uid, closure)
        remaining = {}           # uid -> halves left to emit

        def push_unit(uid, halves):
            remaining[uid] = len(halves)
            for h in halves:
                queue.append((uid, h))

        def pop_one():
            if queue:
                uid, h = queue.popleft()
                h()
                remaining[uid] -= 1

        def drain_until(uid):
            while remaining.get(uid, 0) > 0:
                u2, h = queue.popleft()
                h()
                remaining[u2] -= 1

        def drain_all():
            while queue:
                pop_one()

        # ---- unit builders ----------------------------------------------
        def qk_unit(pair, qk, sc):
            # qk: 0 -> q (wqkv cols 256*pair), 1 -> k (cols 256*pair+128)
            nb = 2 * pair + qk
            dest = qT[pair] if qk == 0 else kT[pair]
            st = {}

            def h1():
                ps = ps5.tile([128, 512], f32, tag="ps5",
                              name=f"qk{pair}_{qk}_{sc}")
                st["ps"] = ps
                for dc in range(4):
                    nc.tensor.matmul(
                        ps[:], lhsT=wqkv[dc][:, 128 * nb:128 * (nb + 1)],
                        rhs=xt[dc][:, 512 * sc:512 * (sc + 1)],
                        start=(dc == 0), stop=False)

            def h2():
                ps = st["ps"]
                for dc in range(4, 8):
                    nc.tensor.matmul(
                        ps[:], lhsT=wqkv[dc][:, 128 * nb:128 * (nb + 1)],
                        rhs=xt[dc][:, 512 * sc:512 * (sc + 1)],
                        start=False, stop=(dc == 7))
                nc.vector.tensor_copy(dest[:, 512 * sc:512 * (sc + 1)],
                                      ps[:])
            return [h1, h2]

        def v_unit(sblk):
            st = {}

            def h1():
                ps = ps5.tile([128, 512], f32, tag="ps5", name=f"v{sblk}")
                st["ps"] = ps
                for dc in range(4):
                    nc.tensor.matmul(
                        ps[:], lhsT=xt[dc][:, 128 * sblk:128 * (sblk + 1)],
                        rhs=wqkv[dc][:, 1024:1536],
                        start=(dc == 0), stop=False)

            def h2():
                ps = st["ps"]
                for dc in range(4, 8):
                    nc.tensor.matmul(
                        ps[:], lhsT=xt[dc][:, 128 * sblk:128 * (sblk + 1)],
                        rhs=wqkv[dc][:, 1024:1536],
                        start=False, stop=(dc == 7))
                nc.vector.tensor_copy(
                    vaug[sblk][:, :, 0:64],
                    ps[:].rearrange("p (h d) -> p h d", h=HL))
            return [h1, h2]

        DRAIN = {"on": False}

        def op_unit(sblk):
            st = {}

            def half(eh):
                def h():
                    if eh == 0:
                        st["osb"] = osbp.tile([128, D], bf, tag="osbp",
                                              name=f"osb{sblk}")
                    osb = st["osb"]
                    if DRAIN["on"]:
                        # scores are done; ping-pong through the idle mm
                        # pool so drain ops pipeline instead of serializing
                        # on the single ps5 bank
                        ps = mm.tile([128, 512], f32, tag="mm",
                                     name=f"op{sblk}_{eh}")
                    else:
                        ps = ps5.tile([128, 512], f32, tag="ps5",
                                      name=f"op{sblk}_{eh}")
                    for cc in range(4):
                        nc.tensor.matmul(
                            ps[:],
                            lhsT=attnT[cc][:, 128 * sblk:128 * (sblk + 1)],
                            rhs=wout[cc][:, 512 * eh:512 * (eh + 1)],
                            start=(cc == 0), stop=(cc == 3))
                    nc.vector.tensor_copy(osb[:, 512 * eh:512 * (eh + 1)],
                                          ps[:])
                    if eh == 1:
                        # partition-split x2 per col-half: 64-row DMAs
                        for q in range(2):
                            for ph in range(2):
                                nc.sync.dma_start(
                                    out_d[128 * sblk + 64 * ph:
                                          128 * sblk + 64 * (ph + 1),
                                          512 * q:512 * (q + 1)],
                                    osb[64 * ph:64 * (ph + 1),
                                        512 * q:512 * (q + 1)])
                return h
            return [half(0), half(1)]

        # ---- attention emitters -----------------------------------------
        def emit_score(pair, ib, jb):
            off = max(0, 128 * (jb - 4 * ib))
            s2 = mm.tile([128, 1024], f32, tag="mm",
                         name=f"s2_{pair}{ib}{jb}")
            for h01 in range(2):
                r0, r1 = 64 * h01, 64 * (h01 + 1)
                nc.tensor.matmul(
                    s2[:, 512 * h01 + off:512 * (h01 + 1)],
                    lhsT=kT[pair][r0:r1, 128 * jb:128 * (jb + 1)],
                    rhs=qT[pair][r0:r1, 512 * ib + off:512 * (ib + 1)],
                    start=True, stop=True)
            pX = pp.tile([128, 1024], bf, tag="pp", name=f"pX{pair}{ib}{jb}")
            s3 = s2[:].rearrange("p (h i) -> p h i", h=2)
            p3 = pX[:].rearrange("p (h i) -> p h i", h=2)
            nc.scalar.activation(p3[:, :, off:512], s3[:, :, off:512],
                                 EXP, scale=0.125)
            if jb >= 4 * ib:
                # in-place causal wedge: keep where i_rel - j >= 0.  Only
                # the first 128 columns past `off` can be masked (j < 128),
                # so restrict the select to that window.
                nc.gpsimd.affine_select(
                    out=p3[:, :, off:off + 128], in_=p3[:, :, off:off + 128],
                    compare_op=GE, fill=0.0, base=0, channel_multiplier=-1,
                    pattern=[[0, 2], [1, 128]])
            return pX

        def emit_av(pair, ib, jb, pX, oA, oB):
            off = max(0, 128 * (jb - 4 * ib))
            n_jb = 4 * (ib + 1)
            for h01, oX in ((0, oA), (1, oB)):
                nc.tensor.matmul(
                    oX[:, off:512],
                    lhsT=vaug[jb][:, 2 * pair + h01, :],
                    rhs=pX[:, 512 * h01 + off:512 * (h01 + 1)],
                    start=(jb == 0), stop=(jb == n_jb - 1))

        def emit_norm(pair, ib, oA, oB):
            for h01, oX in ((0, oA), (1, oB)):
                tmp = rsp.tile([1, 512], f32, tag="rtmp",
                               name=f"rt{pair}{ib}{h01}")
                nc.vector.tensor_copy(tmp[:], oX[64:65, :])
                rs = rsp.tile([1, 512], f32, tag="rsp",
                              name=f"rs{pair}{ib}{h01}")
                nc.vector.reciprocal_approx_fast(rs[:], tmp[:])
                bcs = bcsp.tile([64, 512], f32, tag="bcsp",
                                name=f"bcs{pair}{ib}{h01}")
                nc.gpsimd.partition_broadcast(bcs[:], rs[:])
                nc.vector.tensor_mul(
                    attnT[pair][64 * h01:64 * (h01 + 1),
                                512 * ib:512 * (ib + 1)],
                    oX[0:64, :], bcs[:])

        # ---- build the global filler queue ------------------------------
        # pushed-per-pair so later (ACT-bound) pairs keep some PE filler:
        #   startup: qk(0,*,0);  pair0: v + qk(0,sc>0) + qk(1,sc0)
        #   pair1: qk(1,sc>0) + qk(2,all);  pair2: qk(3,all);  pair3: ops
        def push_qk(pair, sc):
            push_unit(("qk", pair, 0, sc), qk_unit(pair, 0, sc))
            push_unit(("qk", pair, 1, sc), qk_unit(pair, 1, sc))

        push_qk(0, 0)
        pair_pushes = {
            0: lambda: ([push_unit(("v", s), v_unit(s)) for s in range(4)],
                        [(push_qk(0, sc),
                          [push_unit(("v", s), v_unit(s))
                           for s in range(4 * sc, 4 * sc + 4)])
                         for sc in range(1, 4)],
                        push_qk(1, 0)),
            1: lambda: ([push_qk(1, sc) for sc in range(1, 4)],
                        [push_qk(2, sc) for sc in range(4)]),
            2: lambda: [push_qk(3, sc) for sc in range(4)],
            3: lambda: None,
        }

        # ---- main emission: one flat block stream over (pair, ib, jb) ---
        # AV(b) trails score(b+3) so exp latency is always hidden; the
        # stream crosses ib and pair boundaries without flushing, so the
        # last exps of an ib hide behind the next ib's scores.  oA/oB are
        # allocated lazily at first AV flush, keeping <=3 live av banks.
        blocks = []
        for pair in range(PAIRS):
            # pair 3 runs ibs reversed so each ib's out-proj units weave
            # into the next (smaller) ib instead of trailing the kernel
            ib_order = [3, 2, 1, 0] if pair == 3 else [0, 1, 2, 3]
            for ib in ib_order:
                n_jb = 4 * (ib + 1)
                for jb in range(n_jb):
                    blocks.append((pair, ib, jb, jb == n_jb - 1))

        pend = deque()
        cur = {}

        def flush_one():
            pair, ib, jb, last, pX = pend.popleft()
            if (pair, ib) not in cur:
                cur[(pair, ib)] = (
                    av.tile([65, 512], f32, tag="av", name=f"oA{pair}{ib}"),
                    av.tile([65, 512], f32, tag="av", name=f"oB{pair}{ib}"))
            oA, oB = cur[(pair, ib)]
            emit_av(pair, ib, jb, pX, oA, oB)
            if last:
                emit_norm(pair, ib, oA, oB)
                del cur[(pair, ib)]
                if pair == 3:
                    for sblk in range(4 * ib, 4 * ib + 4):
                        push_unit(("op", sblk), op_unit(sblk))

        prev_pair = -1
        for pair, ib, jb, last in blocks:
            if pair != prev_pair:
                pair_pushes[pair]()
                prev_pair = pair
            drain_until(("qk", pair, 0, ib))
            drain_until(("qk", pair, 1, jb // 4))
            if pair == 0:
                drain_until(("v", jb))
            pX = emit_score(pair, ib, jb)
            if len(pend) >= 3:
                flush_one()
            pop_one()
            pend.append((pair, ib, jb, last, pX))
        while pend:
            flush_one()
        DRAIN["on"] = True
        drain_all()

    nc.compile()
    return nc


def _get_nc():
    if "nc" not in _CACHE:
        _CACHE["nc"] = _build()
    return _CACHE["nc"]


def _shard_inputs(x, w_qkv, w_out):
    bf = ml_dtypes.bfloat16
    in_maps = []
    for c in range(N_CORES):
        b, g = divmod(c, 2)
        xT = np.ascontiguousarray(x[b].T).astype(bf)
        wq = w_qkv[512 * g:512 * (g + 1)]
        wk = w_qkv[1024 + 512 * g:1024 + 512 * (g + 1)]
        wv = w_qkv[2048 + 512 * g:2048 + 512 * (g + 1)]
        # device column order: [q0|k0|q1|k1|q2|k2|q3|k3|v] (128 cols each)
        parts = []
        for p in range(4):
            parts.append(wq[128 * p:128 * (p + 1)])
            parts.append(wk[128 * p:128 * (p + 1)])
        parts.append(wv)
        wqkvT = np.ascontiguousarray(
            np.concatenate(parts, axis=0).T).astype(bf)
        woutT = np.ascontiguousarray(w_out[:, 512 * g:512 * (g + 1)].T
                                     ).astype(bf)
        in_maps.append({"xT": xT, "wqkvT": wqkvT, "woutT": woutT})
    return in_maps


def kernel(x, w_qkv, w_out):
    global LAST_EXEC_TIME_NS
    from concourse.bass_utils import run_bass_kernel_spmd

    nc = _get_nc()
    in_maps = _shard_inputs(np.asarray(x, dtype=np.float32),
                            np.asarray(w_qkv, dtype=np.float32),
                            np.asarray(w_out, dtype=np.float32))
    trace = bool(int(os.environ.get("KBENCH_TRACE", "0")))
    res = run_bass_kernel_spmd(nc, in_maps, list(range(N_CORES)), trace=trace)
    LAST_EXEC_TIME_NS = res.exec_time_ns
    out = np.empty((4, S, D), dtype=np.float32)
    for b in range(4):
        out[b] = (res.results[2 * b]["out"].astype(np.float32)
                  + res.results[2 * b + 1]["out"].astype(np.float32))
    return out
